# revision 17
# baseline (speedup 1.0000x reference)
"""Trainium2 Bass kernel for a leaky-integrate-fire (LIF) scan.

Reference computation (forward values only):
    v_t   = mem_{t-1} * 0.25 + x_t          (mem_0 carry = 0)
    s_t   = (v_t > 1.0) ? 1.0 : 0.0         (spike, the output)
    mem_t = (v_t <= 1.0) * v_t              (hard reset)

x: [T=32, B=64, N=16384] f32. Elementwise over (B, N), sequential over T.
Sharding: data-parallel over flattened B*N across 8 cores. Each core's slab
is laid out [P=128, T=32, F=1024] in DRAM.

Design (vs the 2-DVE-op/step + ACT-Sign + int8-store baseline):
- The recurrence carries v (not mem): v_t = reset(v_{t-1})*0.25 + x_t with
  reset(v) = (v<=1) ? v : 0. One custom DVE op (LIF_CHAIN_ANT:
  select(Src0<=1, Src0, 0)*C0 + Src1) does a full step for CV=832 columns,
  so the serial chain is 1 DVE op/step instead of 2.
- For DVE's columns, spikes are extracted AND 2-bit packed by a second
  custom DVE op (LIF_PACK2_ANT: (Src0>1) + 2*(Src1>1)) over step pairs ->
  int8 planes in {0..3}; the host unpacks bits. DVE reads only its own
  column range, so its stream has no cross-engine stalls.
- Pool (gpsimd) chains the last CP=192 columns with a folded carry
  C = 0.25*mem: v = C + x (tt); k4 = (v<=1)*0.25 (one two-scalar ts);
  C' = k4*v (tt) -- 3 Pool-legal ops/step instead of 4. The otherwise-idle
  ACT engine extracts Pool-column spikes as Sign(v-1) int8 per step pair.
  (Packing Pool's columns on DVE or Pool instead would push that engine's
  per-pair pace past the DMA delivery rate and cost more in compute lag
  than the smaller store saves -- measured, not just predicted.)
- All arithmetic stays exact (decay is a power of two; compares/selects
  are exact), so the kernel matches the jax reference bitwise.
- All DMA is issued from the SP ring: one load per step (the compute pace
  ~1.5us/step slightly exceeds the 1.456us/step DMA delivery rate, so
  per-step loads never starve compute after startup; coarser blocks do and
  the lag never recovers since DMA stays saturated), stores trail compute
  in readiness order. The DMA-engine device is the roofline (~360 B/ns in
  the cost model, transfers serialized): in=16.78MB out=2.49MB -> ~53.5us
  of DMA busy with zero gaps + ~2.0us lead-in + ~1.6us trailing sem/drain.
"""

import numpy as np

T = 32
B = 64
N = 16384
NCORES = 8
P = 128                      # SBUF partitions
F = (B // NCORES) * N // P   # 1024 free-dim columns per step per core
CV = 832                     # columns whose chain runs on DVE
CP = F - CV                  # columns whose chain runs on gpsimd (Pool)
NPAIR = T // 2               # packed int8 planes (2 steps/byte)
VSLOTS = 6                   # rotating v tiles
PACK_LAG = 4                 # pack_p issues after chain step 2p+1+PACK_LAG
DECAY = 0.25
VTH = 1.0

_CACHE = {}


def _register_dve_ops():
    """Register the two fused LIF ops in the custom-DVE registry (idempotent).

    Returns (chain_op, pack_op). The uops sha is computed with the same
    lower() that table generation uses, so DveOp.compile()'s pin check
    passes; the numpy reference covers the CoreSim/interp path.
    """
    if "dve_ops" in _CACHE:
        return _CACHE["dve_ops"]
    import concourse.dve_ops as dve_ops
    from concourse.dve_spec import C0, C1, One, Spec, Src0, Src1, Zero, lower, select
    from concourse.dve_uop import DveOpSpec

    chain_spec = Spec(
        # v' = reset(v)*decay + x;  call with s0=DECAY
        body=select(Src0 <= One, Src0, Zero) * C0 + Src1,
        reference=lambda in0, in1, s0, s1, imm2: (
            np.where(in0 <= np.float32(1.0), in0, np.float32(0.0))
            * np.float32(s0)
            + in1
        ).astype(np.float32),
    )
    pack_spec = Spec(
        # b = (v0 > 1) + 2*(v1 > 1);  call with s1=2.0
        body=(Src0 > One) + (Src1 > One) * C1,
        reference=lambda in0, in1, s0, s1, imm2: (
            (in0 > np.float32(1.0)).astype(np.float32)
            + (in1 > np.float32(1.0)).astype(np.float32) * np.float32(s1)
        ).astype(np.float32),
    )

    def _make(name, spec):
        for op in dve_ops.OPS:
            if op.name == name:
                return op
        row = max(dve_ops._SUB_OPCODE_FOR_NAME.values()) + 1
        assert row < 0x20, "custom-DVE byte-36 row field overflow"
        shas = {}
        for ver in ("v3", "v4"):
            try:
                uops = lower(spec, ver=ver)
                shas[ver] = DveOpSpec(
                    name=name, opcode=row, uops=uops, rd1_en=True
                ).sha(ver)
            except Exception:
                pass
        assert shas, f"{name}: lower() failed for every DveVer"
        op = dve_ops.DveOp(name, spec, subdim=False, uops_sha=shas)
        dve_ops.OPS.append(op)
        dve_ops._SUB_OPCODE_FOR_NAME[name] = row
        dve_ops.CUSTOM_DVE_SPECS[name] = spec
        return op

    chain_op = _make("LIF_CHAIN_ANT", chain_spec)
    pack_op = _make("LIF_PACK2_ANT", pack_spec)
    _CACHE["dve_ops"] = (chain_op, pack_op)
    return chain_op, pack_op


def _build_program():
    import concourse.bacc as bacc
    import concourse.tile as tile
    from concourse import mybir

    chain_op, pack_op = _register_dve_ops()

    nc = bacc.Bacc(
        target_bir_lowering=False,
        debug=False,
        enable_asserts=False,
        num_devices=NCORES,
    )
    f32 = mybir.dt.float32
    i8 = mybir.dt.int8
    Alu = mybir.AluOpType
    Act = mybir.ActivationFunctionType

    x_d = nc.dram_tensor("x", [P, T, F], f32, kind="ExternalInput").ap()
    # packed spike planes for cols [0, CV): b_p = s_{2p} + 2*s_{2p+1}
    o_d = nc.dram_tensor("out", [P, NPAIR, CV], i8, kind="ExternalOutput").ap()
    # per-step Sign(v-1) in {-1,0,1} for cols [CV, F)
    s_d = nc.dram_tensor("sgn", [P, T, CP], i8, kind="ExternalOutput").ap()

    # pair ranges per packed store / step ranges per sign store, issued
    # on the sync ring in readiness order (DMA drains them after the loads)
    pk_stores = [(0, 4), (4, 8), (8, 12), (12, 14), (14, 15), (15, 16)]
    sg_stores = [(0, 8), (8, 16), (16, 24), (24, 28), (28, 32)]

    with tile.TileContext(nc) as tc:
        with (
            tc.tile_pool(name="xp", bufs=1) as xpool,
            tc.tile_pool(name="vp", bufs=1) as vpool,
            tc.tile_pool(name="bp", bufs=1) as bpool,
            tc.tile_pool(name="mp", bufs=1) as mpool,
        ):
            xt = xpool.tile([P, T, F], f32)      # whole input, resident
            vt = vpool.tile([P, VSLOTS, F], f32)  # rotating v_t slots
            bt = bpool.tile([P, NPAIR, CV], i8)   # packed spike planes
            st = bpool.tile([P, T, CP], i8)       # Pool-column sign planes
            vseed = mpool.tile([P, CV], f32)      # v_{-1} = 0 for DVE cols
            cg = mpool.tile([P, CP], f32)         # Pool carry C = 0.25*mem
            nbias = mpool.tile([P, 1], f32)       # Sign bias = -VTH
            nc.vector.memset(vseed[:], 0.0)
            nc.gpsimd.memset(cg[:], 0.0)
            nc.vector.memset(nbias[:], -VTH)

            # one load per step: compute pace (~1.5us/step) slightly exceeds
            # the DMA delivery rate (1.456us/step), so with per-step loads
            # the chains never stall after startup. Step 0 goes via the ACT
            # ring: SP's preamble stalls ~0.6us before its first issue while
            # ACT's does not, so this pulls the whole DMA stream forward.
            nc.scalar.dma_start(out=xt[:, 0:1], in_=x_d[:, 0:1, :])
            for t in range(1, T):
                nc.sync.dma_start(out=xt[:, t:t + 1], in_=x_d[:, t:t + 1, :])

            next_pk = 0
            next_sg = 0
            next_pack = 0

            for t in range(T):
                s = t % VSLOTS
                vprev = vseed[:, :] if t == 0 else vt[:, (t - 1) % VSLOTS, :CV]
                # DVE chain, columns [0, CV)
                nc.vector._custom_dve(
                    chain_op,
                    out=vt[:, s, :CV],
                    in0=vprev,
                    in1=xt[:, t, :CV],
                    s0=DECAY,
                )
                # Pool chain, columns [CV, F)
                nc.gpsimd.tensor_tensor(
                    out=vt[:, s, CV:], in0=cg[:], in1=xt[:, t, CV:], op=Alu.add,
                )
                k4 = mpool.tile([P, CP], f32, name="k4")
                nc.gpsimd.tensor_scalar(
                    out=k4[:], in0=vt[:, s, CV:], scalar1=VTH, scalar2=DECAY,
                    op0=Alu.is_le, op1=Alu.mult,
                )
                nc.gpsimd.tensor_tensor(
                    out=cg[:], in0=k4[:], in1=vt[:, s, CV:], op=Alu.mult,
                )
                # lagged spike extraction: pack_p / sign_p issue a few steps
                # after v_{2p+1} exists, well clear of the slot-reuse window
                while next_pack < NPAIR and (
                    t >= 2 * next_pack + 1 + PACK_LAG or t == T - 1
                ):
                    p = next_pack
                    s0 = (2 * p) % VSLOTS  # always even, so s1 = s0 + 1
                    # DVE-column spikes, packed 2 steps/byte
                    nc.vector._custom_dve(
                        pack_op,
                        out=bt[:, p, :],
                        in0=vt[:, s0, :CV],
                        in1=vt[:, s0 + 1, :CV],
                        s1=2.0,
                    )
                    # Pool-column spikes on the idle ACT engine
                    nc.scalar.activation(
                        st[:, 2 * p:2 * p + 2, :], vt[:, s0:s0 + 2, CV:],
                        Act.Sign, bias=nbias[:],
                    )
                    next_pack += 1
                    # stores chase compute in readiness order
                    if next_pk < len(pk_stores) and p + 1 == pk_stores[next_pk][1]:
                        pa, pb = pk_stores[next_pk]
                        nc.sync.dma_start(out=o_d[:, pa:pb, :], in_=bt[:, pa:pb])
                        next_pk += 1
                    if next_sg < len(sg_stores) and 2 * p + 2 == sg_stores[next_sg][1]:
                        sa, sb = sg_stores[next_sg]
                        nc.sync.dma_start(out=s_d[:, sa:sb, :], in_=st[:, sa:sb])
                        next_sg += 1
    nc.compile()
    return nc


def _get_nc():
    if "nc" not in _CACHE:
        _CACHE["nc"] = _build_program()
    return _CACHE["nc"]


def _get_runner():
    """Cache one jitted SPMD executable (same lowering as
    bass_utils.run_bass_kernel_spmd's axon path, which builds a fresh
    jax.jit closure per call and would recompile every time)."""
    if "runner" in _CACHE:
        return _CACHE["runner"]

    import jax
    from jax.sharding import Mesh, PartitionSpec
    from jax.experimental.shard_map import shard_map
    from concourse import bass2jax

    nc = _get_nc()
    bass2jax.install_neuronx_cc_hook()

    # operand order: real inputs, donated output buffers, partition_id last
    in_names = ("x", "out", "sgn", "partition_id")
    out_names = ("out", "sgn")
    out_avals = (
        jax.core.ShapedArray((P, NPAIR, CV), np.int8),
        jax.core.ShapedArray((P, T, CP), np.int8),
    )

    def _body(*args):
        outs = bass2jax._bass_exec_p.bind(
            *args,
            bass2jax.partition_id_tensor(),
            out_avals=out_avals,
            in_names=in_names,
            out_names=out_names,
            lowering_input_output_aliases=(),
            sim_require_finite=True,
            sim_require_nnan=True,
            nc=nc,
        )
        return tuple(outs)

    devices = jax.devices()[:NCORES]
    mesh = Mesh(np.asarray(devices), ("core",))
    sharded = jax.jit(
        shard_map(
            _body,
            mesh=mesh,
            in_specs=(PartitionSpec("core"),) * 3,
            out_specs=(PartitionSpec("core"),) * 2,
            check_rep=False,
        ),
        donate_argnums=(1, 2),
        keep_unused=True,
    )
    _CACHE["runner"] = sharded
    return sharded


def _run_sharded(x_concat):
    """x_concat: [NCORES*P, T, F] host array, core k's slab at rows k*P:(k+1)*P."""
    runner = _get_runner()
    zb = np.zeros((NCORES * P, NPAIR, CV), np.int8)
    zs = np.zeros((NCORES * P, T, CP), np.int8)
    out, sgn = runner(x_concat, zb, zs)
    return np.asarray(out), np.asarray(sgn)


def kernel(x):
    x = np.asarray(x, dtype=np.float32)
    assert x.shape == (T, B, N), x.shape
    # [T, B, N] -> [T, 8, P, F] -> per-core [8, P, T, F] -> concat on axis 0
    x_concat = np.ascontiguousarray(
        x.reshape(T, NCORES, P, F).transpose(1, 2, 0, 3)
    ).reshape(NCORES * P, T, F)
    out, sgn = _run_sharded(x_concat)
    res = np.empty((T, NCORES, P, F), np.float32)
    # cols [0, CV): unpack 2-bit planes b = s_{2p} + 2*s_{2p+1}, b in {0..3}
    raw = out.reshape(NCORES, P, NPAIR, CV)
    bits = np.stack([raw & 1, (raw >> 1) & 1], axis=3)  # [8, P, 16, 2, CV]
    res[:, :, :, :CV] = bits.transpose(2, 3, 0, 1, 4).reshape(T, NCORES, P, CV)
    # cols [CV, F): Sign(v-1) in {-1,0,1}; spike iff raw == 1 (v > 1 exactly)
    sg = sgn.reshape(NCORES, P, T, CP)
    res[:, :, :, CV:] = (sg == 1).transpose(2, 0, 1, 3)
    return res.reshape(T, B, N)


# revision 18
# speedup vs baseline: 1.0016x; 1.0016x over previous
"""Trainium2 Bass kernel for a leaky-integrate-fire (LIF) scan.

Reference computation (forward values only):
    v_t   = mem_{t-1} * 0.25 + x_t          (mem_0 carry = 0)
    s_t   = (v_t > 1.0) ? 1.0 : 0.0         (spike, the output)
    mem_t = (v_t <= 1.0) * v_t              (hard reset)

x: [T=32, B=64, N=16384] f32. Elementwise over (B, N), sequential over T.
Sharding: data-parallel over flattened B*N across 8 cores. Each core's slab
is laid out [P=128, T=32, F=1024] in DRAM.

Design (vs the 2-DVE-op/step + ACT-Sign + int8-store baseline):
- The recurrence carries v (not mem): v_t = reset(v_{t-1})*0.25 + x_t with
  reset(v) = (v<=1) ? v : 0. One custom DVE op (LIF_CHAIN_ANT:
  select(Src0<=1, Src0, 0)*C0 + Src1) does a full step for CV=832 columns,
  so the serial chain is 1 DVE op/step instead of 2.
- For DVE's columns, spikes are extracted AND 2-bit packed by a second
  custom DVE op (LIF_PACK2_ANT: (Src0>1) + 2*(Src1>1)) over step pairs ->
  int8 planes in {0..3}; the host unpacks bits. DVE reads only its own
  column range, so its stream has no cross-engine stalls.
- Pool (gpsimd) chains the last CP=192 columns with a folded carry
  C = 0.25*mem: v = C + x (tt); k4 = (v<=1)*0.25 (one two-scalar ts);
  C' = k4*v (tt) -- 3 Pool-legal ops/step instead of 4. The otherwise-idle
  ACT engine extracts Pool-column spikes as Sign(v-1) int8 per step pair.
  (Packing Pool's columns on DVE or Pool instead would push that engine's
  per-pair pace past the DMA delivery rate and cost more in compute lag
  than the smaller store saves -- measured, not just predicted.)
- All arithmetic stays exact (decay is a power of two; compares/selects
  are exact), so the kernel matches the jax reference bitwise.
- All DMA is issued from the SP ring: one load per step (the compute pace
  ~1.5us/step slightly exceeds the 1.456us/step DMA delivery rate, so
  per-step loads never starve compute after startup; coarser blocks do and
  the lag never recovers since DMA stays saturated), stores trail compute
  in readiness order. The DMA-engine device is the roofline (~360 B/ns in
  the cost model, transfers serialized): in=16.78MB out=2.49MB -> ~53.5us
  of DMA busy with zero gaps + ~2.0us lead-in + ~1.6us trailing sem/drain.
"""

import numpy as np

T = 32
B = 64
N = 16384
NCORES = 8
P = 128                      # SBUF partitions
F = (B // NCORES) * N // P   # 1024 free-dim columns per step per core
CV = 832                     # columns whose chain runs on DVE
CP = F - CV                  # columns whose chain runs on gpsimd (Pool)
NPAIR = T // 2               # packed int8 planes (2 steps/byte)
VSLOTS = 6                   # rotating v tiles
PACK_LAG = 4                 # pack_p issues after chain step 2p+1+PACK_LAG
DECAY = 0.25
VTH = 1.0

_CACHE = {}


def _register_dve_ops():
    """Register the two fused LIF ops in the custom-DVE registry (idempotent).

    Returns (chain_op, pack_op). The uops sha is computed with the same
    lower() that table generation uses, so DveOp.compile()'s pin check
    passes; the numpy reference covers the CoreSim/interp path.
    """
    if "dve_ops" in _CACHE:
        return _CACHE["dve_ops"]
    import concourse.dve_ops as dve_ops
    from concourse.dve_spec import C0, C1, One, Spec, Src0, Src1, Zero, lower, select
    from concourse.dve_uop import DveOpSpec

    chain_spec = Spec(
        # v' = reset(v)*decay + x;  call with s0=DECAY
        body=select(Src0 <= One, Src0, Zero) * C0 + Src1,
        reference=lambda in0, in1, s0, s1, imm2: (
            np.where(in0 <= np.float32(1.0), in0, np.float32(0.0))
            * np.float32(s0)
            + in1
        ).astype(np.float32),
    )
    pack_spec = Spec(
        # b = (v0 > 1) + 2*(v1 > 1);  call with s1=2.0
        body=(Src0 > One) + (Src1 > One) * C1,
        reference=lambda in0, in1, s0, s1, imm2: (
            (in0 > np.float32(1.0)).astype(np.float32)
            + (in1 > np.float32(1.0)).astype(np.float32) * np.float32(s1)
        ).astype(np.float32),
    )

    def _make(name, spec):
        for op in dve_ops.OPS:
            if op.name == name:
                return op
        row = max(dve_ops._SUB_OPCODE_FOR_NAME.values()) + 1
        assert row < 0x20, "custom-DVE byte-36 row field overflow"
        shas = {}
        for ver in ("v3", "v4"):
            try:
                uops = lower(spec, ver=ver)
                shas[ver] = DveOpSpec(
                    name=name, opcode=row, uops=uops, rd1_en=True
                ).sha(ver)
            except Exception:
                pass
        assert shas, f"{name}: lower() failed for every DveVer"
        op = dve_ops.DveOp(name, spec, subdim=False, uops_sha=shas)
        dve_ops.OPS.append(op)
        dve_ops._SUB_OPCODE_FOR_NAME[name] = row
        dve_ops.CUSTOM_DVE_SPECS[name] = spec
        return op

    chain_op = _make("LIF_CHAIN_ANT", chain_spec)
    pack_op = _make("LIF_PACK2_ANT", pack_spec)
    _CACHE["dve_ops"] = (chain_op, pack_op)
    return chain_op, pack_op


def _build_program():
    import concourse.bacc as bacc
    import concourse.tile as tile
    from concourse import mybir

    chain_op, pack_op = _register_dve_ops()

    nc = bacc.Bacc(
        target_bir_lowering=False,
        debug=False,
        enable_asserts=False,
        num_devices=NCORES,
    )
    f32 = mybir.dt.float32
    i8 = mybir.dt.int8
    Alu = mybir.AluOpType
    Act = mybir.ActivationFunctionType

    x_d = nc.dram_tensor("x", [P, T, F], f32, kind="ExternalInput").ap()
    # packed spike planes for cols [0, CV): b_p = s_{2p} + 2*s_{2p+1}
    o_d = nc.dram_tensor("out", [P, NPAIR, CV], i8, kind="ExternalOutput").ap()
    # per-step Sign(v-1) in {-1,0,1} for cols [CV, F)
    s_d = nc.dram_tensor("sgn", [P, T, CP], i8, kind="ExternalOutput").ap()

    # pair ranges per packed store / step ranges per sign store, issued
    # on the sync ring in readiness order (DMA drains them after the loads)
    pk_stores = [(0, 4), (4, 8), (8, 12), (12, 14), (14, 15), (15, 16)]
    sg_stores = [(0, 8), (8, 16), (16, 24), (24, 28), (28, 32)]

    with tile.TileContext(nc) as tc:
        with (
            tc.tile_pool(name="xp", bufs=1) as xpool,
            tc.tile_pool(name="vp", bufs=1) as vpool,
            tc.tile_pool(name="bp", bufs=1) as bpool,
            tc.tile_pool(name="mp", bufs=1) as mpool,
        ):
            xt = xpool.tile([P, T, F], f32)      # whole input, resident
            vt = vpool.tile([P, VSLOTS, F], f32)  # rotating v_t slots
            bt = bpool.tile([P, NPAIR, CV], i8)   # packed spike planes
            st = bpool.tile([P, T, CP], i8)       # Pool-column sign planes
            vseed = mpool.tile([P, CV], f32)      # v_{-1} = 0 for DVE cols
            cg = mpool.tile([P, CP], f32)         # Pool carry C = 0.25*mem
            nbias = mpool.tile([P, 1], f32)       # Sign bias = -VTH
            nc.vector.memset(vseed[:], 0.0)
            nc.gpsimd.memset(cg[:], 0.0)
            nc.vector.memset(nbias[:], -VTH)

            # one load per step: compute pace (~1.5us/step) slightly exceeds
            # the DMA delivery rate (1.456us/step), so with per-step loads
            # the chains never stall after startup
            for t in range(T):
                nc.sync.dma_start(out=xt[:, t:t + 1], in_=x_d[:, t:t + 1, :])

            next_pk = 0
            next_sg = 0
            next_pack = 0

            for t in range(T):
                s = t % VSLOTS
                vprev = vseed[:, :] if t == 0 else vt[:, (t - 1) % VSLOTS, :CV]
                # DVE chain, columns [0, CV)
                nc.vector._custom_dve(
                    chain_op,
                    out=vt[:, s, :CV],
                    in0=vprev,
                    in1=xt[:, t, :CV],
                    s0=DECAY,
                )
                # Pool chain, columns [CV, F)
                nc.gpsimd.tensor_tensor(
                    out=vt[:, s, CV:], in0=cg[:], in1=xt[:, t, CV:], op=Alu.add,
                )
                k4 = mpool.tile([P, CP], f32, name="k4")
                nc.gpsimd.tensor_scalar(
                    out=k4[:], in0=vt[:, s, CV:], scalar1=VTH, scalar2=DECAY,
                    op0=Alu.is_le, op1=Alu.mult,
                )
                nc.gpsimd.tensor_tensor(
                    out=cg[:], in0=k4[:], in1=vt[:, s, CV:], op=Alu.mult,
                )
                # lagged spike extraction: pack_p / sign_p issue a few steps
                # after v_{2p+1} exists, well clear of the slot-reuse window
                while next_pack < NPAIR and (
                    t >= 2 * next_pack + 1 + PACK_LAG or t == T - 1
                ):
                    p = next_pack
                    s0 = (2 * p) % VSLOTS  # always even, so s1 = s0 + 1
                    # DVE-column spikes, packed 2 steps/byte
                    nc.vector._custom_dve(
                        pack_op,
                        out=bt[:, p, :],
                        in0=vt[:, s0, :CV],
                        in1=vt[:, s0 + 1, :CV],
                        s1=2.0,
                    )
                    # Pool-column spikes on the idle ACT engine
                    nc.scalar.activation(
                        st[:, 2 * p:2 * p + 2, :], vt[:, s0:s0 + 2, CV:],
                        Act.Sign, bias=nbias[:],
                    )
                    next_pack += 1
                    # stores chase compute in readiness order
                    if next_pk < len(pk_stores) and p + 1 == pk_stores[next_pk][1]:
                        pa, pb = pk_stores[next_pk]
                        nc.sync.dma_start(out=o_d[:, pa:pb, :], in_=bt[:, pa:pb])
                        next_pk += 1
                    if next_sg < len(sg_stores) and 2 * p + 2 == sg_stores[next_sg][1]:
                        sa, sb = sg_stores[next_sg]
                        nc.sync.dma_start(out=s_d[:, sa:sb, :], in_=st[:, sa:sb])
                        next_sg += 1
    nc.compile()
    return nc


def _get_nc():
    if "nc" not in _CACHE:
        _CACHE["nc"] = _build_program()
    return _CACHE["nc"]


def _get_runner():
    """Cache one jitted SPMD executable (same lowering as
    bass_utils.run_bass_kernel_spmd's axon path, which builds a fresh
    jax.jit closure per call and would recompile every time)."""
    if "runner" in _CACHE:
        return _CACHE["runner"]

    import jax
    from jax.sharding import Mesh, PartitionSpec
    from jax.experimental.shard_map import shard_map
    from concourse import bass2jax

    nc = _get_nc()
    bass2jax.install_neuronx_cc_hook()

    # operand order: real inputs, donated output buffers, partition_id last
    in_names = ("x", "out", "sgn", "partition_id")
    out_names = ("out", "sgn")
    out_avals = (
        jax.core.ShapedArray((P, NPAIR, CV), np.int8),
        jax.core.ShapedArray((P, T, CP), np.int8),
    )

    def _body(*args):
        outs = bass2jax._bass_exec_p.bind(
            *args,
            bass2jax.partition_id_tensor(),
            out_avals=out_avals,
            in_names=in_names,
            out_names=out_names,
            lowering_input_output_aliases=(),
            sim_require_finite=True,
            sim_require_nnan=True,
            nc=nc,
        )
        return tuple(outs)

    devices = jax.devices()[:NCORES]
    mesh = Mesh(np.asarray(devices), ("core",))
    sharded = jax.jit(
        shard_map(
            _body,
            mesh=mesh,
            in_specs=(PartitionSpec("core"),) * 3,
            out_specs=(PartitionSpec("core"),) * 2,
            check_rep=False,
        ),
        donate_argnums=(1, 2),
        keep_unused=True,
    )
    _CACHE["runner"] = sharded
    return sharded


def _run_sharded(x_concat):
    """x_concat: [NCORES*P, T, F] host array, core k's slab at rows k*P:(k+1)*P."""
    runner = _get_runner()
    zb = np.zeros((NCORES * P, NPAIR, CV), np.int8)
    zs = np.zeros((NCORES * P, T, CP), np.int8)
    out, sgn = runner(x_concat, zb, zs)
    return np.asarray(out), np.asarray(sgn)


def kernel(x):
    x = np.asarray(x, dtype=np.float32)
    assert x.shape == (T, B, N), x.shape
    # [T, B, N] -> [T, 8, P, F] -> per-core [8, P, T, F] -> concat on axis 0
    x_concat = np.ascontiguousarray(
        x.reshape(T, NCORES, P, F).transpose(1, 2, 0, 3)
    ).reshape(NCORES * P, T, F)
    out, sgn = _run_sharded(x_concat)
    res = np.empty((T, NCORES, P, F), np.float32)
    # cols [0, CV): unpack 2-bit planes b = s_{2p} + 2*s_{2p+1}, b in {0..3}
    raw = out.reshape(NCORES, P, NPAIR, CV)
    bits = np.stack([raw & 1, (raw >> 1) & 1], axis=3)  # [8, P, 16, 2, CV]
    res[:, :, :, :CV] = bits.transpose(2, 3, 0, 1, 4).reshape(T, NCORES, P, CV)
    # cols [CV, F): Sign(v-1) in {-1,0,1}; spike iff raw == 1 (v > 1 exactly)
    sg = sgn.reshape(NCORES, P, T, CP)
    res[:, :, :, CV:] = (sg == 1).transpose(2, 0, 1, 3)
    return res.reshape(T, B, N)
